# revision 7
# baseline (speedup 1.0000x reference)
"""ConfidenceGate Trainium2 kernel (8 NeuronCores, SPMD).

Problem recap (shapes hardcoded from the spec):
  x:      (4, 512, 256, 7, 7) f32
  prev_x: (4, 512, 256, 7, 7) f32
  match:  (4, 512, 513) f32
  + tiny proj/LN/MLP params.
Reference returns c[0] -> (512, 1): only batch 0 contributes to the output.

Strategy (v2):
  * Only batch 0 is computed; data-parallel over M=512 rows: 8 cores x 64.
  * top1 = argmax(match[0,:,:512]) on host; prev rows pre-gathered per shard.
  * Host stores both big streams CHANNEL-MAJOR fp16, spatial padded 49->50:
    dram[c_half, roi*50 + s].  The on-device grouped reduce (DVE, fp16)
    directly produces the proj-matmul lhsT (c, roi) -- no transposes or
    deinterleaves anywhere in the streaming path.  1/49 is folded into the
    weights; the rhs carries an extra NEGATED column-mean column so the
    psum holds [v | -mu] and centering is one ACT bias-add.
  * All transcendentals use ONE activation table set (natural_log + exp):
    1/sqrt(q) = exp(-0.5 ln q), sigmoid(z) = 1/(1+exp(-z)) via DVE recip.
  * match is fp16 too; stats spread over gpsimd (TT/TS only -- no STT
    there) + ACT, keeping DVE for pooling reduces + the few accum ops.
  * MLP hidden state pre-accumulated over the 4 early features during
    streaming (ACT per-partition-scale copies + Pool adds); the tail only
    folds in cos_sim on DVE.
"""

import sys

if "/opt/trn_rl_repo" not in sys.path:
    sys.path.insert(0, "/opt/trn_rl_repo")

import numpy as np

B, M, N, C, G = 4, 512, 512, 256, 7
S = G * G                      # 49 spatial positions
SP = 50                        # padded spatial (4B-aligned fp16 groups)
PP, HH = 32, 32                # proj dim, MLP hidden
NCORES = 8
MS = M // NCORES               # 64 rows per core
CH = C // 2                    # 128 channels per half = partition dim
COLS = MS * SP                 # 3200 cols per half
EPS = 1e-9
LN_EPS = 1e-5
NEG = -60000.0                 # fp16-safe "-inf" for second-max masking

# roi sub-chunk splits: (which, half, roi_off, roi_len); v first, x last,
# with the final sync-ring chunk tapered so the last reduce is small.
CHUNKS = [
    ("v", 0, 0, 32), ("v", 0, 32, 32),
    ("v", 1, 0, 32), ("v", 1, 32, 32),
    ("x", 0, 0, 32), ("x", 0, 32, 32),
    ("x", 1, 0, 16), ("x", 1, 16, 16),
]
# x half-1 rois 32..63 ride the scalar ring (issued after mt/aux, so they
# land mid-stream and their reduces run early)
CHUNKS_SC = [("x", 1, 32, 16), ("x", 1, 48, 16)]

# auxf (f32, 64 partitions) column layout
A_PB = 0          # psum preload [proj_b | -mean(proj_b)] (64, 33)
A_LG = 33         # ln_g replicated (64, 32)
A_LB = 65         # ln_b replicated (64, 32)
A_W1 = 97         # w1 replicated: block f at [97+32f : 129+32f), f=0..4
A_B1 = 257        # b1 replicated (64, 32)
A_W2 = 289        # w2[0] replicated (64, 32)
A_NB2 = 321       # -b2 replicated (64, 1)
A_COLS = 322

_CACHE = {}


def _build():
    import concourse.bacc as bacc
    import concourse.tile as tile
    import concourse.mybir as mybir

    dt = mybir.dt
    Alu = mybir.AluOpType
    Act = mybir.ActivationFunctionType
    Ax = mybir.AxisListType
    f32 = dt.float32
    f16 = dt.float16

    nc = bacc.Bacc("TRN2", target_bir_lowering=False, debug=False)

    xs_d = nc.dram_tensor("xs", [CH, 2 * COLS], f16, kind="ExternalInput")
    pv_d = nc.dram_tensor("pv", [CH, 2 * COLS], f16, kind="ExternalInput")
    mt_d = nc.dram_tensor("mt", [MS, N + 1], f16, kind="ExternalInput")
    a16_d = nc.dram_tensor("a16", [CH, 66], f16, kind="ExternalInput")
    axf_d = nc.dram_tensor("axf", [MS, A_COLS], f32, kind="ExternalInput")
    out_d = nc.dram_tensor("out", [MS, 1], f32, kind="ExternalOutput")

    with tile.TileContext(nc) as tc, nc.allow_low_precision(
        reason="fp16 pooling sums; |sum|<=~30, rel err ~1e-3 vs 2e-2 gate"
    ):
        with (
            tc.tile_pool(name="persist", bufs=1) as per,
            tc.tile_pool(name="chunks", bufs=1) as big,
            tc.tile_pool(name="scratch", bufs=1) as scr,
            tc.tile_pool(name="psum", bufs=1, space="PSUM") as psp,
        ):
            # ---- small loads on the scalar (ACT) HWDGE ring ----
            mt = per.tile([MS, N + 1], f16)
            nc.scalar.dma_start(out=mt[:], in_=mt_d[:])
            a16 = per.tile([CH, 66], f16)
            nc.scalar.dma_start(out=a16[:], in_=a16_d[:])
            axf = per.tile([MS, A_COLS], f32)
            nc.scalar.dma_start(out=axf[:], in_=axf_d[:])

            # ---- constants + hoist the (single) ACT table load to t=0 ----
            e9 = per.tile([MS, 1], f32)
            nc.gpsimd.memset(e9[:], EPS)
            eln = per.tile([MS, 1], f32)
            nc.gpsimd.memset(eln[:], LN_EPS)
            dmy = per.tile([1, 1], f32)
            nc.gpsimd.memset(dmy[:], 1.0)
            pre = scr.tile([1, 2], f32, tag="pre")
            nc.scalar.activation(pre[:, 0:1], dmy[:], Act.Ln, bias=e9[0:1, 0:1])
            nc.scalar.activation(pre[:, 1:2], dmy[:], Act.Exp)

            # ---- big chunked loads ----
            src = {"x": xs_d, "v": pv_d}
            ctiles = {}
            for w, h, ro, rl in CHUNKS:
                t = big.tile([CH, rl * SP], f16, tag=f"ch_{w}{h}_{ro}",
                             name=f"ch_{w}{h}_{ro}")
                co = h * COLS + ro * SP
                nc.sync.dma_start(out=t[:], in_=src[w][:, co:co + rl * SP])
                ctiles[(w, h, ro)] = t
            for w, h, ro, rl in CHUNKS_SC:
                t = big.tile([CH, rl * SP], f16, tag=f"ch_{w}{h}_{ro}",
                             name=f"ch_{w}{h}_{ro}")
                co = h * COLS + ro * SP
                nc.scalar.dma_start(out=t[:], in_=src[w][:, co:co + rl * SP])
                ctiles[(w, h, ro)] = t

            real = mt[:, 0:N]
            pd = mt[:, N:N + 1]

            # ---- match stats (overlapped with streaming) ----
            rmass = per.tile([MS, 1], f32)
            jr = scr.tile([MS, N], f16, tag="jk")
            nc.scalar.activation(jr[:], real, Act.Copy, accum_out=rmass[:])
            lnr = per.tile([MS, N], f16)
            nc.scalar.activation(lnr[:], real, Act.Ln, bias=e9[:])
            # p_max [DVE, fp16 2x]
            f_pmax = per.tile([MS, 1], f32)
            nc.vector.reduce_max(f_pmax[:], real, axis=Ax.X)
            # second max via mask-out [gpsimd] + DVE reduce
            eqm = scr.tile([MS, N], f16, tag="jk3")
            nc.gpsimd.tensor_scalar(eqm[:], real, f_pmax[:], None,
                                    op0=Alu.is_equal)
            nmsk = scr.tile([MS, N], f16, tag="jk4")
            nc.gpsimd.tensor_scalar(nmsk[:], eqm[:], NEG, None, op0=Alu.mult)
            msk = scr.tile([MS, N], f16, tag="jk5")
            nc.gpsimd.tensor_tensor(msk[:], nmsk[:], real, op=Alu.add)
            m2 = per.tile([MS, 1], f32)
            nc.vector.reduce_max(m2[:], msk[:], axis=Ax.X)
            f_gap = per.tile([MS, 1], f32)
            nc.gpsimd.tensor_tensor(f_gap[:], f_pmax[:], m2[:], op=Alu.subtract)
            # entropy: f_ent = sum(real * ln(real+eps)) = -ent
            je = scr.tile([MS, N], f16, tag="jk2")
            nc.gpsimd.tensor_tensor(je[:], real, lnr[:], op=Alu.mult)
            f_ent = per.tile([MS, 1], f32)
            je2 = scr.tile([MS, N], f16, tag="jk6")
            nc.scalar.activation(je2[:], je[:], Act.Copy, accum_out=f_ent[:])
            # feat0 = 1 - p_dummy [DVE]
            f_pd = per.tile([MS, 1], f32)
            nc.vector.tensor_scalar(f_pd[:], pd, -1.0, 1.0, op0=Alu.mult,
                                    op1=Alu.add)
            # masks [gpsimd]
            hr9 = per.tile([MS, 1], f32)
            nc.gpsimd.tensor_scalar(hr9[:], rmass[:], EPS, None, op0=Alu.is_gt)
            hr6 = per.tile([MS, 1], f32)
            nc.gpsimd.tensor_scalar(hr6[:], rmass[:], 1e-6, None, op0=Alu.is_gt)

            # ---- MLP hidden pre-accumulation over the 4 early features ----
            # tmp_f = w1[:,f]*feat_f  (ACT per-partition scale), summed on Pool
            tmps = []
            for i, fap in enumerate((f_pd, f_pmax, f_gap, f_ent)):
                t = scr.tile([MS, HH], f32, tag=f"tmpf{i}")
                nc.scalar.activation(
                    t[:], axf[:, A_W1 + i * HH:A_W1 + (i + 1) * HH],
                    Act.Copy, scale=fap[:])
                tmps.append(t)
            hAc = per.tile([MS, HH], f32, tag="hA")
            nc.gpsimd.tensor_tensor(hAc[:], tmps[0][:], axf[:, A_B1:A_B1 + HH],
                                    op=Alu.add)
            hBc = per.tile([MS, HH], f32, tag="hB")
            nc.gpsimd.tensor_tensor(hBc[:], hAc[:], tmps[1][:], op=Alu.add)
            hCc = per.tile([MS, HH], f32, tag="hC")
            nc.gpsimd.tensor_tensor(hCc[:], hBc[:], tmps[2][:], op=Alu.add)
            hDc = per.tile([MS, HH], f32, tag="hD")
            nc.gpsimd.tensor_tensor(hDc[:], hCc[:], tmps[3][:], op=Alu.add)

            # ---- proj psum tiles preloaded with [proj_b | -mean(proj_b)] ----
            vps = {}
            for w in ("x", "v"):
                t = psp.tile([MS, PP + 1], f32, tag=f"vps_{w}", name=f"vps_{w}")
                nc.scalar.activation(t[:], axf[:, A_PB:A_PB + PP + 1], Act.Copy)
                vps[w] = t

            # ---- streaming: grouped pool reduce -> P_h; matmul per half ----
            P_t = {}
            for w in ("x", "v"):
                for h in (0, 1):
                    P_t[(w, h)] = per.tile([CH, MS], f16, tag=f"P_{w}{h}",
                                           name=f"P_{w}{h}")
            nchunks = {}
            for w, h, ro, rl in CHUNKS + CHUNKS_SC:
                nchunks[(w, h)] = nchunks.get((w, h), 0) + 1
            done = {}
            for w, h, ro, rl in CHUNKS + CHUNKS_SC:
                ct = ctiles[(w, h, ro)]
                P = P_t[(w, h)]
                nc.vector.reduce_sum(
                    P[:, ro:ro + rl],
                    ct[:].rearrange("p (r s) -> p r s", s=SP), axis=Ax.X)
                done[(w, h)] = done.get((w, h), 0) + 1
                if done[(w, h)] == nchunks[(w, h)]:
                    nc.tensor.matmul(
                        vps[w][:], P[:], a16[:, h * 33:(h + 1) * 33],
                        start=False, stop=(h == 1), skip_group_check=True)

            # ---- layernorm chains (v overlapped with x streaming) ----
            ys = {}
            for w in ("v", "x"):
                vp = vps[w]
                # center: v + (-mu)  (DVE per-partition scalar add from PSUM)
                ctr = per.tile([MS, PP], f32, tag=f"ctr_{w}")
                nc.vector.tensor_scalar(ctr[:], vp[:, 0:PP], vp[:, PP:PP + 1],
                                        None, op0=Alu.add)
                jv = scr.tile([MS, PP], f32, tag=f"jv_{w}")
                vsum = per.tile([MS, 1], f32, tag=f"vs_{w}")
                nc.scalar.activation(jv[:], ctr[:], Act.Square,
                                     accum_out=vsum[:])
                lnv = scr.tile([MS, 1], f32, tag=f"lnv_{w}")
                nc.scalar.activation(lnv[:], vsum[:], Act.Ln, scale=1.0 / PP,
                                     bias=eln[:])
                rs = per.tile([MS, 1], f32, tag=f"rs_{w}")
                nc.scalar.activation(rs[:], lnv[:], Act.Exp, scale=-0.5)
                # g*rstd on Pool, then y = ctr*(g*rstd) + b on Pool
                gr = scr.tile([MS, PP], f32, tag=f"gr_{w}")
                nc.gpsimd.tensor_scalar(gr[:], axf[:, A_LG:A_LG + PP], rs[:],
                                        None, op0=Alu.mult)
                yg = scr.tile([MS, PP], f32, tag=f"yg_{w}")
                nc.gpsimd.tensor_tensor(yg[:], ctr[:], gr[:], op=Alu.mult)
                y = per.tile([MS, PP], f32, tag=f"y_{w}")
                nc.gpsimd.tensor_tensor(y[:], yg[:], axf[:, A_LB:A_LB + PP],
                                        op=Alu.add)
                ys[w] = y

            # ---- cosine similarity ----
            yx, yv = ys["x"], ys["v"]
            nrm = per.tile([MS, 2], f32)
            jn = scr.tile([MS, PP], f32, tag="jn")
            nc.vector.scalar_tensor_tensor(
                jn[:], yv[:], 1.0, yv[:], op0=Alu.mult, op1=Alu.mult,
                accum_out=nrm[:, 1:2])
            jn2 = scr.tile([MS, PP], f32, tag="jn2")
            nc.vector.scalar_tensor_tensor(
                jn2[:], yx[:], 1.0, yx[:], op0=Alu.mult, op1=Alu.mult,
                accum_out=nrm[:, 0:1])
            dot = per.tile([MS, 1], f32)
            jn3 = scr.tile([MS, PP], f32, tag="jn3")
            nc.vector.scalar_tensor_tensor(
                jn3[:], yx[:], 1.0, yv[:], op0=Alu.mult, op1=Alu.mult,
                accum_out=dot[:])
            q = per.tile([MS, 1], f32)
            nc.vector.tensor_tensor(q[:], nrm[:, 0:1], nrm[:, 1:2], op=Alu.mult)
            lnq = scr.tile([MS, 1], f32, tag="lnq")
            nc.scalar.activation(lnq[:], q[:], Act.Ln)
            rq = per.tile([MS, 1], f32)
            nc.scalar.activation(rq[:], lnq[:], Act.Exp, scale=-0.5)
            f_cos = per.tile([MS, 1], f32)
            nc.vector.scalar_tensor_tensor(
                f_cos[:], dot[:], rq[:], hr9[:], op0=Alu.mult, op1=Alu.mult)

            # ---- finish MLP on DVE ----
            hE = per.tile([MS, HH], f32, tag="hE")
            nc.vector.scalar_tensor_tensor(
                hE[:], axf[:, A_W1 + 4 * HH:A_W1 + 5 * HH], f_cos[:], hDc[:],
                op0=Alu.mult, op1=Alu.add)
            hR = per.tile([MS, HH], f32, tag="hR")
            nc.vector.tensor_scalar(hR[:], hE[:], 0.0, None, op0=Alu.max)
            logit = per.tile([MS, 1], f32)
            jl = scr.tile([MS, HH], f32, tag="jl")
            nc.vector.scalar_tensor_tensor(
                jl[:], hR[:], 1.0, axf[:, A_W2:A_W2 + HH], op0=Alu.mult,
                op1=Alu.mult, accum_out=logit[:])
            # sigmoid = 1/(1+exp(-z));  exp(-logit - b2)
            ez = per.tile([MS, 1], f32)
            nc.scalar.activation(ez[:], logit[:], Act.Exp, scale=-1.0,
                                 bias=axf[:, A_NB2:A_NB2 + 1])
            den = per.tile([MS, 1], f32)
            nc.vector.tensor_scalar(den[:], ez[:], 1.0, None, op0=Alu.add)
            sg = per.tile([MS, 1], f32)
            nc.vector.reciprocal(sg[:], den[:])
            gt = per.tile([MS, 1], f32)
            nc.vector.scalar_tensor_tensor(
                gt[:], sg[:], 0.999, hr6[:], op0=Alu.min, op1=Alu.mult)
            res = per.tile([MS, 1], f32)
            nc.vector.tensor_scalar(res[:], gt[:], 0.001, None, op0=Alu.max)
            nc.sync.dma_start(out=out_d[:], in_=res[:])

    nc.finalize()
    return nc


def _get_nc():
    if "nc" not in _CACHE:
        _CACHE["nc"] = _build()
    return _CACHE["nc"]


def _pack_stream(rows_f32):
    """(64, 256*49) f32 -> (128, 6400) f16 channel-major padded layout."""
    f16 = np.float16
    t = rows_f32.reshape(MS, C, S).transpose(1, 0, 2)      # (256, 64, 49)
    buf = np.zeros((C, MS, SP), dtype=f16)
    buf[:, :, :S] = t
    out = np.empty((CH, 2 * COLS), dtype=f16)
    out[:, :COLS] = buf[0:CH].reshape(CH, COLS)
    out[:, COLS:] = buf[CH:C].reshape(CH, COLS)
    return out


def make_in_maps(x, prev_x, match, proj_w, proj_b, ln_g, ln_b, w1, b1, w2, b2):
    f32 = np.float32
    f16 = np.float16
    x0 = np.asarray(x[0], dtype=f32).reshape(M, C, S)
    p0 = np.asarray(prev_x[0], dtype=f32).reshape(N, C, S)
    mt0 = np.ascontiguousarray(np.asarray(match[0], dtype=f32))
    real0 = mt0[:, :N]
    rm = real0.sum(axis=1)
    top1 = np.where(rm > EPS, np.argmax(real0, axis=1), 0)

    proj_w = np.asarray(proj_w, dtype=f32)   # (32, 256)
    proj_b = np.asarray(proj_b, dtype=f32)
    # a16 per half h: [c, 0:32] = proj_w[:, 128h+c].T/49 ; col 32 = -rowmean/32
    a16 = np.zeros((CH, 66), dtype=f16)
    for h in (0, 1):
        blk = proj_w[:, h * CH:(h + 1) * CH].T / S       # (128, 32)
        a16[:, h * 33:h * 33 + PP] = blk
        a16[:, h * 33 + PP] = -blk.mean(axis=1)
    axf = np.zeros((MS, A_COLS), dtype=f32)
    axf[:, A_PB:A_PB + PP] = proj_b
    axf[:, A_PB + PP] = -proj_b.mean()
    axf[:, A_LG:A_LG + PP] = np.asarray(ln_g, dtype=f32)
    axf[:, A_LB:A_LB + PP] = np.asarray(ln_b, dtype=f32)
    w1 = np.asarray(w1, dtype=f32)           # (32, 5)
    for f in range(5):
        axf[:, A_W1 + f * HH:A_W1 + (f + 1) * HH] = w1[:, f]
    axf[:, A_B1:A_B1 + HH] = np.asarray(b1, dtype=f32)
    axf[:, A_W2:A_W2 + HH] = np.asarray(w2, dtype=f32)[0]
    axf[:, A_NB2] = -np.asarray(b2, dtype=f32)[0]

    in_maps = []
    for i in range(NCORES):
        lo, hi = i * MS, (i + 1) * MS
        in_maps.append({
            "xs": _pack_stream(x0[lo:hi]),
            "pv": _pack_stream(p0[top1[lo:hi]]),
            "mt": np.ascontiguousarray(mt0[lo:hi]).astype(f16),
            "a16": a16, "axf": axf,
        })
    return in_maps


def run(in_maps, trace=False):
    from concourse.bass_utils import run_bass_kernel_spmd
    res = run_bass_kernel_spmd(_get_nc(), in_maps, list(range(NCORES)), trace=trace)
    out = np.concatenate(
        [res.results[i]["out"].reshape(MS, 1) for i in range(NCORES)], axis=0)
    return out.astype(np.float32), res


def kernel(x, prev_x, match, proj_w, proj_b, ln_g, ln_b, w1, b1, w2, b2):
    in_maps = make_in_maps(x, prev_x, match, proj_w, proj_b, ln_g, ln_b, w1, b1, w2, b2)
    out, _ = run(in_maps, trace=False)
    return out
